# revision 28
# baseline (speedup 1.0000x reference)
"""Masked attention (B=2, H=8, S=4096, D=64) on 8 Trainium2 NeuronCores.

Sharding: batch*head parallel. Core c owns flat heads {2c, 2c+1} (same batch
index b = c // 4 for both, so the [S, S] mask is shared by both heads of a
core).

Device algorithm (per core, per head), transposed so no on-chip transposes are
ever needed; the exp() of the softmax is SPLIT between the ScalarE (true exp)
and the VectorE (Schraudolph bit-trick exp) because ScalarE's 1 elem/lane/cycle
throughput on 33.5M elements/core (~219us) is otherwise the hard bottleneck:

  - Host supplies K^T augmented with a ones row as [65, S] fp16, Q^T pre-scaled
    by A*SCALE with a B row as [65, S] fp16, so the score matmul directly
    produces y[k, q] = A*x + B in PSUM, where x = (q . k)/sqrt(D) is the true
    logit, A = 1024/ln2 and B = 1024*(15 - c_rms). V is augmented with a ones
    column as [S, 65] fp16 (row 64 of the AV output = softmax denominator) and
    laid out chunk-major per partition so its DMA is contiguous.
  - Scores are computed transposed: y[k, q] via matmul(lhsT=K^T chunk [65,128],
    rhs=Q^T block [65, 512]); fp16 streams 1 column/cycle and keeps the HAM
    clock at 2.4 GHz.
  - Per score tile (a (3-chunk group, head) pair), one of two drain paths:
      ACT path: ScalarE activation computes pt = exp(y/A - B/A) = exp(x)
        (PSUM->SBUF fp16), then VectorE multiplies by the keep-mask (fp16 2x).
      DVE path (~27% of tiles): one fused VectorE tensor_mul with int16 output:
        i16 = convert(y * maskT). Bit-cast as fp16 this is Schraudolph's
        approximate exp (rel err ~1.7% RMS), and masked entries are exactly
        0x0000 = 0.0. One 1x-rate pass does drain+mask+exp, freeing ScalarE.
    No max-subtraction is needed: scores ~ N(0,1), exp stays in range.
  - AV accumulates transposed-free: matmul(lhsT=[V|1] chunk [128,65],
    rhs=P^T chunk [128,512], fp16) accumulates out^T[d,q] over the 32 k-chunks
    in PSUM; row 64 = softmax denominator. Each slot's AV matmuls are emitted
    AFTER the next slot's score matmuls (PE executes matmuls strictly in
    order, and AV depends on the softmax chain: emitting AV one slot late
    keeps the score stream ahead of ScalarE/VectorE at q-block boundaries).
  - Finished AV accumulators go PSUM -> DRAM directly by DMA; the host divides
    rows 0:64 by row 64 and transposes to [S, 64] during unshard.
"""

from contextlib import ExitStack

import numpy as np

import concourse.tile as tile
from concourse import bacc, mybir
from concourse.bass_utils import run_bass_kernel_spmd

B, H, S, D = 2, 8, 4096, 64
N_CORES = 8
HPC = (B * H) // N_CORES  # heads per core = 2
SCALE = 1.0 / 8.0  # 1/sqrt(D)

# Schraudolph constants for fp16 (10-bit mantissa, bias 15):
#   i16 = round(A*x + B); bitcast(i16) ~= exp(x), rel err ~1.7% RMS.
# A = 1024/ln2; B = 1024*(15 - c) with c ~= 0.0573 (RMS-optimal), rounded so
# B is exactly representable in fp16 (it is sent as a Q^T row).
A_CONST = 1477.3197218702985
B_CONST = 15304.0

F32 = mybir.dt.float32
BF16 = mybir.dt.bfloat16
F16 = mybir.dt.float16
I16 = mybir.dt.int16


def build_kernel_body(tc, qT, kT, vaug, maskT, outT, s=S, hpc=HPC, qb_size=512,
                      group_size=2, psum_s_bufs=3, pt_bufs=8, mask_bufs=8,
                      dve_period=18, dve_slots=(1, 5, 9, 12, 16),
                      gp_tt_every=6, mask_lookahead=5, av_defer=3):
    """Emit the attention program. All APs are DRAM tensors:
    qT, kT: [hpc, 65, s] f16; vaug: [hpc, 128, n_chunks*65] f16;
    maskT: [s, s] f16; outT: [hpc, 65, s] f32.
    """
    nc = tc.nc
    n_qb = s // qb_size
    n_chunks = s // 128
    groups = []
    c0 = 0
    while c0 < n_chunks:
        groups.append((c0, min(group_size, n_chunks - c0)))
        c0 += group_size

    ctx = ExitStack()
    const = ctx.enter_context(tc.tile_pool(name="const", bufs=1))
    mask_pool = ctx.enter_context(tc.tile_pool(name="mask", bufs=mask_bufs))
    pt_pool = ctx.enter_context(tc.tile_pool(name="pt", bufs=pt_bufs))
    out_pool = ctx.enter_context(tc.tile_pool(name="osb", bufs=4))
    psum_s_pool = ctx.enter_context(
        tc.tile_pool(name="psum_s", bufs=psum_s_bufs, space="PSUM"))
    psum_av_pool = ctx.enter_context(
        tc.tile_pool(name="psum_av", bufs=hpc, space="PSUM"))

    # Resident tensors: Q^T, K^T (fp16, 65 rows: d + affine row), V|1 chunked.
    qT_sb = const.tile([D + 1, hpc, s], F16)
    kT_sb = const.tile([D + 1, hpc, s], F16)
    vaug_sb = const.tile([128, hpc, n_chunks, D + 1], F16)
    # Per-partition bias for the ACT path: exp(y/A - B/A) = exp(x).
    bias_sb = const.tile([128, 1], F32)
    nc.gpsimd.memset(bias_sb[:, :], -B_CONST / A_CONST)

    # Prologue DMAs, ordered and split by first-use time: per-head prefixes
    # feed the first score matmuls; mask tiles feed the first drains; V feeds
    # the first AV groups; qT remainders are only needed from q-block 1
    # (~30us in), so they go last.
    g0w = groups[0][1] * 128
    n_pre_masks = min(6, len(groups))
    for h in range(hpc):
        nc.sync.dma_start(out=kT_sb[:, h, 0:g0w], in_=kT[h, :, 0:g0w])
        nc.sync.dma_start(out=qT_sb[:, h, 0:qb_size], in_=qT[h, :, 0:qb_size])

    # Masks stream on the GpSimd software-DGE queue so their issue ops never
    # serialize behind the big const loads on the Sync queue. A lookahead of
    # `mask_lookahead` groups keeps transfers ahead of consumption even when
    # GpSimd also runs offloaded mask-multiplies.
    mask_plan = []  # (qb, gi) in consumption order
    for qb_ in range(n_qb):
        for gi_ in range(len(groups)):
            mask_plan.append((qb_, gi_))
    mask_tiles = {}
    mask_next = [0]

    def issue_masks(upto):
        while mask_next[0] < min(upto, len(mask_plan)):
            qb_, gi_ = mask_plan[mask_next[0]]
            c0_, gs_ = groups[gi_]
            qs_ = slice(qb_ * qb_size, (qb_ + 1) * qb_size)
            mt = mask_pool.tile([128, group_size, qb_size], F16)
            nc.gpsimd.dma_start(
                out=mt[:, :gs_, :],
                in_=maskT[c0_ * 128:(c0_ + gs_) * 128, qs_].rearrange(
                    "(c p) q -> p c q", p=128),
            )
            mask_tiles[(qb_, gi_)] = mt
            mask_next[0] += 1

    # HAM warm-up: ~12 fp16 matmuls on a memset tile, needing no DMA — they
    # span the prologue DMA debt and bring the PE clock to 2.4 GHz before the
    # first real score matmul issues.
    warm = const.tile([128, qb_size], F16)
    nc.vector.memset(warm, 0.0)
    wp = psum_s_pool.tile([128, group_size, qb_size], F32, name="wp", tag="ps")
    for _ in range(12):
        nc.tensor.matmul(wp[:, 0, :], lhsT=warm[:, 0:128], rhs=warm[:, :],
                         start=True, stop=True)

    issue_masks(n_pre_masks)
    # Const loads in consumption order on the Sync queue: kT front half,
    # V (needed from the first AV group), kT back half. The qT remainders
    # (only needed from q-block 1, ~30us in) are issued from inside the slot
    # loop so they never compete with the early mask stream.
    vaug_r = [vaug[h, :, :].rearrange("p (c w) -> p c w", w=D + 1)
              for h in range(hpc)]
    kmid = s // 2
    for h in range(hpc):
        nc.sync.dma_start(out=kT_sb[:, h, g0w:kmid], in_=kT[h, :, g0w:kmid])
    for h in range(hpc):
        nc.sync.dma_start(out=vaug_sb[:, h, :, :], in_=vaug_r[h])
    for h in range(hpc):
        nc.sync.dma_start(out=kT_sb[:, h, kmid:], in_=kT[h, :, kmid:])

    # Flat slot schedule: (qb, group, head). AV for slot i is emitted during
    # slot i+1, after that slot's score matmuls.
    slots = []
    for qb in range(n_qb):
        for gi, (c0_, gs_) in enumerate(groups):
            for h in range(hpc):
                slots.append((qb, gi, c0_, gs_, h))

    av_cur = {}  # h -> (tile, qb, qs)

    def flush_av(h):
        # Drain a finished accumulator: PSUM -> SBUF (alternating engines to
        # balance the two near-critical drain engines), then DMA to DRAM.
        avt, _, qs_ = av_cur[h]
        osb = out_pool.tile([D + 1, qb_size], F32, name="osb")
        if h == 0:
            nc.vector.tensor_copy(osb[:, :], avt[:, :])
        else:
            nc.scalar.copy(osb[:, :], avt[:, :])
        nc.sync.dma_start(out=outT[h, :, qs_], in_=osb[:, :])

    def emit_av(qb, c0_, gs_, h, pt, qs):
        cur = av_cur.get(h)
        if cur is None or cur[1] != qb:
            if cur is not None:
                flush_av(h)
            avt = psum_av_pool.tile([D + 1, qb_size], F32, tag="av", name="av")
            av_cur[h] = (avt, qb, qs)
        avt = av_cur[h][0]
        for j in range(gs_):
            c = c0_ + j
            nc.tensor.matmul(
                avt[:, :],
                lhsT=vaug_sb[:, h, c, :],
                rhs=pt[:, j, :],
                start=(c == 0),
                stop=(c == n_chunks - 1),
            )

    deferred = []
    pending_tt = []
    act_count = 0
    for si, (qb, gi, c0_, gs_, h) in enumerate(slots):
        qs = slice(qb * qb_size, (qb + 1) * qb_size)
        if h == 0:
            gidx = qb * len(groups) + gi
            issue_masks(gidx + 1 + mask_lookahead)
            if si == 20:
                for h_ in range(hpc):
                    if qb_size < s:
                        nc.sync.dma_start(out=qT_sb[:, h_, qb_size:],
                                          in_=qT[h_, :, qb_size:])
        mt = mask_tiles[(qb, gi)]

        ps = psum_s_pool.tile([128, group_size, qb_size], F32)
        for j in range(gs_):
            c = c0_ + j
            nc.tensor.matmul(
                ps[:, j, :],
                lhsT=kT_sb[:, h, c * 128:(c + 1) * 128],
                rhs=qT_sb[:, h, qs],
                start=True,
                stop=True,
            )

        pt = pt_pool.tile([128, group_size, qb_size], F16)
        is_dve = (si % dve_period) in dve_slots
        if is_dve:
            # Fused drain+mask+exp on VectorE: i16 = convert(y*mask); the fp16
            # bit pattern of i16 = round(A*x+B) approximates exp(x); mask=0
            # gives exactly 0.0. Emitted ahead of the previous ACT slot's
            # mask-multiply so it runs concurrently with that ACTIVATE on the
            # in-order Vector queue.
            nc.vector.tensor_mul(
                pt[:, :gs_, :].bitcast(I16), ps[:, :gs_, :], mt[:, :gs_, :])
        else:
            nc.scalar.activation(
                pt[:, :gs_, :], ps[:, :gs_, :],
                mybir.ActivationFunctionType.Exp,
                scale=1.0 / A_CONST, bias=bias_sb[:, :],
            )
        # Previous ACT slot's mask-multiply: deferred one slot so this slot's
        # fused DVE drain (if any) sits ahead of it on the Vector queue.
        if pending_tt:
            opt, omt, ogs, use_gp = pending_tt.pop()
            eng = nc.gpsimd if use_gp else nc.vector
            eng.tensor_mul(opt[:, :ogs, :], opt[:, :ogs, :], omt[:, :ogs, :])
        if not is_dve:
            use_gp = bool(gp_tt_every) and (
                act_count % gp_tt_every == gp_tt_every - 1)
            act_count += 1
            pending_tt.append((pt, mt, gs_, use_gp))

        deferred.append((qb, c0_, gs_, h, pt, qs))
        if len(deferred) > av_defer:
            emit_av(*deferred.pop(0))
    if pending_tt:
        opt, omt, ogs, use_gp = pending_tt.pop()
        eng = nc.gpsimd if use_gp else nc.vector
        eng.tensor_mul(opt[:, :ogs, :], opt[:, :ogs, :], omt[:, :ogs, :])
    while deferred:
        emit_av(*deferred.pop(0))
    for h in range(hpc):
        flush_av(h)
    ctx.close()


def build_nc(s=S, hpc=HPC, **kwargs):
    nc = bacc.Bacc(
        "TRN2",
        target_bir_lowering=False,
        debug=False,
        num_devices=N_CORES,
    )
    n_chunks = s // 128
    qT = nc.dram_tensor("qT", [hpc, D + 1, s], F16, kind="ExternalInput").ap()
    kT = nc.dram_tensor("kT", [hpc, D + 1, s], F16, kind="ExternalInput").ap()
    vaug = nc.dram_tensor(
        "vaug", [hpc, 128, n_chunks * (D + 1)], F16, kind="ExternalInput").ap()
    maskT = nc.dram_tensor("maskT", [s, s], F16, kind="ExternalInput").ap()
    outT = nc.dram_tensor("outT", [hpc, D + 1, s], F32, kind="ExternalOutput").ap()
    with tile.TileContext(nc) as tc:
        build_kernel_body(tc, qT, kT, vaug, maskT, outT, s=s, hpc=hpc, **kwargs)
    nc.compile()
    return nc


_NC_CACHE = {}


def get_nc(**kwargs):
    key = tuple(sorted(kwargs.items()))
    if key not in _NC_CACHE:
        _NC_CACHE[key] = build_nc(**kwargs)
    return _NC_CACHE[key]


def make_in_maps(query, key, value, self_attn_mask):
    """Host-side shard + layout prep. Returns list of 8 per-core input dicts."""
    q = np.asarray(query, dtype=np.float32)
    k = np.asarray(key, dtype=np.float32)
    v = np.asarray(value, dtype=np.float32)
    m = np.asarray(self_attn_mask)
    n_chunks = S // 128
    in_maps = []
    ones = np.ones((S, 1), np.float32)
    qscale = np.float32(A_CONST * SCALE)
    for core in range(N_CORES):
        flats = [HPC * core + i for i in range(HPC)]
        pairs = [(f // H, f % H) for f in flats]
        b = pairs[0][0]
        qT = np.stack([
            np.concatenate([q[b_, h_].T * qscale,
                            np.full((1, S), B_CONST, np.float32)], axis=0)
            for b_, h_ in pairs]).astype(np.float16)
        kT = np.stack([
            np.concatenate([k[b_, h_].T, np.ones((1, S), np.float32)], axis=0)
            for b_, h_ in pairs]).astype(np.float16)
        # [S,65] -> chunk-major per partition: [128, n_chunks*65] contiguous.
        vaug = np.stack([
            np.concatenate([v[b_, h_], ones], axis=1)
            .reshape(n_chunks, 128, D + 1).transpose(1, 0, 2)
            .reshape(128, n_chunks * (D + 1))
            for b_, h_ in pairs]).astype(np.float16)
        maskT = np.ascontiguousarray(
            (~m[b, 0]).T).astype(np.float16)
        in_maps.append({
            "qT": np.ascontiguousarray(qT),
            "kT": np.ascontiguousarray(kT),
            "vaug": np.ascontiguousarray(vaug),
            "maskT": maskT,
        })
    return in_maps


def gather_output(results):
    out = np.empty((B, H, S, D), np.float32)
    for core, r in enumerate(results):
        oT = r["outT"].astype(np.float32)  # [HPC, 65, S]
        for i in range(HPC):
            f = HPC * core + i
            b_, h_ = f // H, f % H
            out[b_, h_] = (oT[i, :D, :] / oT[i, D:D + 1, :]).T
    return out


def kernel(query, key, value, self_attn_mask, trace=False, tmpdir=None,
           **build_kwargs):
    nc = get_nc(**build_kwargs)
    in_maps = make_in_maps(query, key, value, self_attn_mask)
    kwargs = {"tmpdir": tmpdir} if tmpdir else {}
    res = run_bass_kernel_spmd(nc, in_maps, core_ids=list(range(N_CORES)),
                               trace=trace, **kwargs)
    out = gather_output(res.results)
    if trace:
        kernel.last_result = res
    return out


# revision 30
# speedup vs baseline: 1.1151x; 1.1151x over previous
"""Masked attention (B=2, H=8, S=4096, D=64) on 8 Trainium2 NeuronCores.

Sharding: batch*head parallel. Core c owns flat heads {2c, 2c+1} (same batch
index b = c // 4 for both, so the [S, S] mask is shared by both heads of a
core).

Device algorithm (per core, per head), transposed so no on-chip transposes are
ever needed; the exp() of the softmax is SPLIT between the ScalarE (true exp)
and the VectorE (Schraudolph bit-trick exp) because ScalarE's 1 elem/lane/cycle
throughput on 33.5M elements/core (~219us) is otherwise the hard bottleneck:

  - Host supplies K^T augmented with a ones row as [65, S] fp16, Q^T pre-scaled
    by A*SCALE with a B row as [65, S] fp16, so the score matmul directly
    produces y[k, q] = A*x + B in PSUM, where x = (q . k)/sqrt(D) is the true
    logit, A = 1024/ln2 and B = 1024*(15 - c_rms). V is augmented with a ones
    column as [S, 65] fp16 (row 64 of the AV output = softmax denominator) and
    laid out chunk-major per partition so its DMA is contiguous.
  - Scores are computed transposed: y[k, q] via matmul(lhsT=K^T chunk [65,128],
    rhs=Q^T block [65, 512]); fp16 streams 1 column/cycle and keeps the HAM
    clock at 2.4 GHz.
  - Per score tile (a (3-chunk group, head) pair), one of two drain paths:
      ACT path: ScalarE activation computes pt = exp(y/A - B/A) = exp(x)
        (PSUM->SBUF fp16), then VectorE multiplies by the keep-mask (fp16 2x).
      DVE path (~27% of tiles): one fused VectorE tensor_mul with int16 output:
        i16 = convert(y * maskT). Bit-cast as fp16 this is Schraudolph's
        approximate exp (rel err ~1.7% RMS), and masked entries are exactly
        0x0000 = 0.0. One 1x-rate pass does drain+mask+exp, freeing ScalarE.
    No max-subtraction is needed: scores ~ N(0,1), exp stays in range.
  - AV accumulates transposed-free: matmul(lhsT=[V|1] chunk [128,65],
    rhs=P^T chunk [128,512], fp16) accumulates out^T[d,q] over the 32 k-chunks
    in PSUM; row 64 = softmax denominator. Each slot's AV matmuls are emitted
    AFTER the next slot's score matmuls (PE executes matmuls strictly in
    order, and AV depends on the softmax chain: emitting AV one slot late
    keeps the score stream ahead of ScalarE/VectorE at q-block boundaries).
  - Finished AV accumulators go PSUM -> DRAM directly by DMA; the host divides
    rows 0:64 by row 64 and transposes to [S, 64] during unshard.
"""

from contextlib import ExitStack

import numpy as np

import concourse.tile as tile
from concourse import bacc, mybir
from concourse.bass_utils import run_bass_kernel_spmd

B, H, S, D = 2, 8, 4096, 64
N_CORES = 8
HPC = (B * H) // N_CORES  # heads per core = 2
SCALE = 1.0 / 8.0  # 1/sqrt(D)

# Schraudolph constants for fp16 (10-bit mantissa, bias 15):
#   i16 = round(A*x + B); bitcast(i16) ~= exp(x), rel err ~1.7% RMS.
# A = 1024/ln2; B = 1024*(15 - c) with c ~= 0.0573 (RMS-optimal), rounded so
# B is exactly representable in fp16 (it is sent as a Q^T row).
A_CONST = 1477.3197218702985
B_CONST = 15304.0

F32 = mybir.dt.float32
BF16 = mybir.dt.bfloat16
F16 = mybir.dt.float16
I16 = mybir.dt.int16


def build_kernel_body(tc, qT, kT, vaug, maskT, outT, s=S, hpc=HPC, qb_size=512,
                      group_size=2, psum_s_bufs=3, pt_bufs=8, mask_bufs=8,
                      dve_period=18, dve_slots=(1, 5, 9, 12, 16),
                      gp_tt_every=0, mask_lookahead=4, av_defer=2):
    """Emit the attention program. All APs are DRAM tensors:
    qT, kT: [hpc, 65, s] f16; vaug: [hpc, 128, n_chunks*65] f16;
    maskT: [s, s] f16; outT: [hpc, 65, s] f32.
    """
    nc = tc.nc
    n_qb = s // qb_size
    n_chunks = s // 128
    groups = []
    c0 = 0
    while c0 < n_chunks:
        groups.append((c0, min(group_size, n_chunks - c0)))
        c0 += group_size

    ctx = ExitStack()
    const = ctx.enter_context(tc.tile_pool(name="const", bufs=1))
    mask_pool = ctx.enter_context(tc.tile_pool(name="mask", bufs=mask_bufs))
    pt_pool = ctx.enter_context(tc.tile_pool(name="pt", bufs=pt_bufs))
    out_pool = ctx.enter_context(tc.tile_pool(name="osb", bufs=4))
    psum_s_pool = ctx.enter_context(
        tc.tile_pool(name="psum_s", bufs=psum_s_bufs, space="PSUM"))
    psum_av_pool = ctx.enter_context(
        tc.tile_pool(name="psum_av", bufs=hpc, space="PSUM"))

    # Resident tensors: Q^T, K^T (fp16, 65 rows: d + affine row), V|1 chunked.
    qT_sb = const.tile([D + 1, hpc, s], F16)
    kT_sb = const.tile([D + 1, hpc, s], F16)
    vaug_sb = const.tile([128, hpc, n_chunks, D + 1], F16)
    # Per-partition bias for the ACT path: exp(y/A - B/A) = exp(x).
    bias_sb = const.tile([128, 1], F32)
    nc.gpsimd.memset(bias_sb[:, :], -B_CONST / A_CONST)
    # Dummy 1-element exp so the ~2.7us ACT_TABLE_LOAD happens during the
    # prologue DMA debt instead of delaying the first real exp.
    tl_sb = const.tile([128, 1], F32)
    nc.scalar.activation(tl_sb[:, :], bias_sb[:, :],
                         mybir.ActivationFunctionType.Exp)

    # Prologue DMAs, ordered and split by first-use time: per-head prefixes
    # feed the first score matmuls; mask tiles feed the first drains; V feeds
    # the first AV groups; qT remainders are only needed from q-block 1
    # (~30us in), so they go last.
    g0w = groups[0][1] * 128
    n_pre_masks = min(6, len(groups))
    for h in range(hpc):
        nc.sync.dma_start(out=kT_sb[:, h, 0:g0w], in_=kT[h, :, 0:g0w])
        nc.sync.dma_start(out=qT_sb[:, h, 0:qb_size], in_=qT[h, :, 0:qb_size])

    # Masks stream on the GpSimd software-DGE queue so their issue ops never
    # serialize behind the big const loads on the Sync queue. A lookahead of
    # `mask_lookahead` groups keeps transfers ahead of consumption even when
    # GpSimd also runs offloaded mask-multiplies.
    mask_plan = []  # (qb, gi) in consumption order
    for qb_ in range(n_qb):
        for gi_ in range(len(groups)):
            mask_plan.append((qb_, gi_))
    mask_tiles = {}
    mask_next = [0]

    def issue_masks(upto):
        while mask_next[0] < min(upto, len(mask_plan)):
            qb_, gi_ = mask_plan[mask_next[0]]
            c0_, gs_ = groups[gi_]
            qs_ = slice(qb_ * qb_size, (qb_ + 1) * qb_size)
            mt = mask_pool.tile([128, group_size, qb_size], F16)
            nc.gpsimd.dma_start(
                out=mt[:, :gs_, :],
                in_=maskT[c0_ * 128:(c0_ + gs_) * 128, qs_].rearrange(
                    "(c p) q -> p c q", p=128),
            )
            mask_tiles[(qb_, gi_)] = mt
            mask_next[0] += 1

    # HAM warm-up: ~12 fp16 matmuls on a memset tile, needing no DMA — they
    # span the prologue DMA debt and bring the PE clock to 2.4 GHz before the
    # first real score matmul issues.
    warm = const.tile([128, qb_size], F16)
    nc.vector.memset(warm, 0.0)
    wp = psum_s_pool.tile([128, group_size, qb_size], F32, name="wp", tag="ps")
    for _ in range(12):
        nc.tensor.matmul(wp[:, 0, :], lhsT=warm[:, 0:128], rhs=warm[:, :],
                         start=True, stop=True)

    issue_masks(n_pre_masks)
    # Const loads in consumption order on the Sync queue: kT front half,
    # V (needed from the first AV group), kT back half. The qT remainders
    # (only needed from q-block 1, ~30us in) are issued from inside the slot
    # loop so they never compete with the early mask stream.
    vaug_r = [vaug[h, :, :].rearrange("p (c w) -> p c w", w=D + 1)
              for h in range(hpc)]
    kmid = s // 2
    for h in range(hpc):
        nc.sync.dma_start(out=kT_sb[:, h, g0w:kmid], in_=kT[h, :, g0w:kmid])
    for h in range(hpc):
        nc.sync.dma_start(out=vaug_sb[:, h, :, :], in_=vaug_r[h])
    for h in range(hpc):
        nc.sync.dma_start(out=kT_sb[:, h, kmid:], in_=kT[h, :, kmid:])

    # Flat slot schedule: (qb, group, head). AV for slot i is emitted during
    # slot i+1, after that slot's score matmuls.
    slots = []
    for qb in range(n_qb):
        for gi, (c0_, gs_) in enumerate(groups):
            for h in range(hpc):
                slots.append((qb, gi, c0_, gs_, h))

    av_cur = {}  # h -> (tile, qb, qs)

    def flush_av(h):
        # Drain a finished accumulator: PSUM -> SBUF (alternating engines to
        # balance the two near-critical drain engines), then DMA to DRAM.
        avt, _, qs_ = av_cur[h]
        osb = out_pool.tile([D + 1, qb_size], F32, name="osb")
        if h == 0:
            nc.vector.tensor_copy(osb[:, :], avt[:, :])
        else:
            nc.scalar.copy(osb[:, :], avt[:, :])
        nc.sync.dma_start(out=outT[h, :, qs_], in_=osb[:, :])

    def emit_av(qb, c0_, gs_, h, pt, qs):
        cur = av_cur.get(h)
        if cur is None or cur[1] != qb:
            if cur is not None:
                flush_av(h)
            avt = psum_av_pool.tile([D + 1, qb_size], F32, tag="av", name="av")
            av_cur[h] = (avt, qb, qs)
        avt = av_cur[h][0]
        for j in range(gs_):
            c = c0_ + j
            nc.tensor.matmul(
                avt[:, :],
                lhsT=vaug_sb[:, h, c, :],
                rhs=pt[:, j, :],
                start=(c == 0),
                stop=(c == n_chunks - 1),
            )

    deferred = []
    pending_tt = []
    act_count = 0
    for si, (qb, gi, c0_, gs_, h) in enumerate(slots):
        qs = slice(qb * qb_size, (qb + 1) * qb_size)
        if h == 0:
            gidx = qb * len(groups) + gi
            issue_masks(gidx + 1 + mask_lookahead)
            if si == 20:
                for h_ in range(hpc):
                    if qb_size < s:
                        nc.sync.dma_start(out=qT_sb[:, h_, qb_size:],
                                          in_=qT[h_, :, qb_size:])
        mt = mask_tiles[(qb, gi)]

        ps = psum_s_pool.tile([128, group_size, qb_size], F32)
        for j in range(gs_):
            c = c0_ + j
            nc.tensor.matmul(
                ps[:, j, :],
                lhsT=kT_sb[:, h, c * 128:(c + 1) * 128],
                rhs=qT_sb[:, h, qs],
                start=True,
                stop=True,
            )

        pt = pt_pool.tile([128, group_size, qb_size], F16)
        is_dve = (si % dve_period) in dve_slots
        if is_dve:
            # Fused drain+mask+exp on VectorE: i16 = convert(y*mask); the fp16
            # bit pattern of i16 = round(A*x+B) approximates exp(x); mask=0
            # gives exactly 0.0. Emitted ahead of the previous ACT slot's
            # mask-multiply so it runs concurrently with that ACTIVATE on the
            # in-order Vector queue.
            nc.vector.tensor_mul(
                pt[:, :gs_, :].bitcast(I16), ps[:, :gs_, :], mt[:, :gs_, :])
        else:
            nc.scalar.activation(
                pt[:, :gs_, :], ps[:, :gs_, :],
                mybir.ActivationFunctionType.Exp,
                scale=1.0 / A_CONST, bias=bias_sb[:, :],
            )
        # Previous ACT slot's mask-multiply: deferred one slot so this slot's
        # fused DVE drain (if any) sits ahead of it on the Vector queue.
        if pending_tt:
            opt, omt, ogs, use_gp = pending_tt.pop()
            eng = nc.gpsimd if use_gp else nc.vector
            eng.tensor_mul(opt[:, :ogs, :], opt[:, :ogs, :], omt[:, :ogs, :])
        if not is_dve:
            use_gp = bool(gp_tt_every) and (
                act_count % gp_tt_every == gp_tt_every - 1)
            act_count += 1
            pending_tt.append((pt, mt, gs_, use_gp))

        deferred.append((qb, c0_, gs_, h, pt, qs))
        if len(deferred) > av_defer:
            emit_av(*deferred.pop(0))
    if pending_tt:
        opt, omt, ogs, use_gp = pending_tt.pop()
        eng = nc.gpsimd if use_gp else nc.vector
        eng.tensor_mul(opt[:, :ogs, :], opt[:, :ogs, :], omt[:, :ogs, :])
    while deferred:
        emit_av(*deferred.pop(0))
    for h in range(hpc):
        flush_av(h)
    ctx.close()


def build_nc(s=S, hpc=HPC, **kwargs):
    nc = bacc.Bacc(
        "TRN2",
        target_bir_lowering=False,
        debug=False,
        num_devices=N_CORES,
    )
    n_chunks = s // 128
    qT = nc.dram_tensor("qT", [hpc, D + 1, s], F16, kind="ExternalInput").ap()
    kT = nc.dram_tensor("kT", [hpc, D + 1, s], F16, kind="ExternalInput").ap()
    vaug = nc.dram_tensor(
        "vaug", [hpc, 128, n_chunks * (D + 1)], F16, kind="ExternalInput").ap()
    maskT = nc.dram_tensor("maskT", [s, s], F16, kind="ExternalInput").ap()
    outT = nc.dram_tensor("outT", [hpc, D + 1, s], F32, kind="ExternalOutput").ap()
    with tile.TileContext(nc) as tc:
        build_kernel_body(tc, qT, kT, vaug, maskT, outT, s=s, hpc=hpc, **kwargs)
    nc.compile()
    return nc


_NC_CACHE = {}


def get_nc(**kwargs):
    key = tuple(sorted(kwargs.items()))
    if key not in _NC_CACHE:
        _NC_CACHE[key] = build_nc(**kwargs)
    return _NC_CACHE[key]


def make_in_maps(query, key, value, self_attn_mask):
    """Host-side shard + layout prep. Returns list of 8 per-core input dicts."""
    q = np.asarray(query, dtype=np.float32)
    k = np.asarray(key, dtype=np.float32)
    v = np.asarray(value, dtype=np.float32)
    m = np.asarray(self_attn_mask)
    n_chunks = S // 128
    in_maps = []
    ones = np.ones((S, 1), np.float32)
    qscale = np.float32(A_CONST * SCALE)
    for core in range(N_CORES):
        flats = [HPC * core + i for i in range(HPC)]
        pairs = [(f // H, f % H) for f in flats]
        b = pairs[0][0]
        qT = np.stack([
            np.concatenate([q[b_, h_].T * qscale,
                            np.full((1, S), B_CONST, np.float32)], axis=0)
            for b_, h_ in pairs]).astype(np.float16)
        kT = np.stack([
            np.concatenate([k[b_, h_].T, np.ones((1, S), np.float32)], axis=0)
            for b_, h_ in pairs]).astype(np.float16)
        # [S,65] -> chunk-major per partition: [128, n_chunks*65] contiguous.
        vaug = np.stack([
            np.concatenate([v[b_, h_], ones], axis=1)
            .reshape(n_chunks, 128, D + 1).transpose(1, 0, 2)
            .reshape(128, n_chunks * (D + 1))
            for b_, h_ in pairs]).astype(np.float16)
        maskT = np.ascontiguousarray(
            (~m[b, 0]).T).astype(np.float16)
        in_maps.append({
            "qT": np.ascontiguousarray(qT),
            "kT": np.ascontiguousarray(kT),
            "vaug": np.ascontiguousarray(vaug),
            "maskT": maskT,
        })
    return in_maps


def gather_output(results):
    out = np.empty((B, H, S, D), np.float32)
    for core, r in enumerate(results):
        oT = r["outT"].astype(np.float32)  # [HPC, 65, S]
        for i in range(HPC):
            f = HPC * core + i
            b_, h_ = f // H, f % H
            out[b_, h_] = (oT[i, :D, :] / oT[i, D:D + 1, :]).T
    return out


def kernel(query, key, value, self_attn_mask, trace=False, tmpdir=None,
           **build_kwargs):
    nc = get_nc(**build_kwargs)
    in_maps = make_in_maps(query, key, value, self_attn_mask)
    kwargs = {"tmpdir": tmpdir} if tmpdir else {}
    res = run_bass_kernel_spmd(nc, in_maps, core_ids=list(range(N_CORES)),
                               trace=trace, **kwargs)
    out = gather_output(res.results)
    if trace:
        kernel.last_result = res
    return out


# revision 31
# speedup vs baseline: 1.1256x; 1.0094x over previous
"""Masked attention (B=2, H=8, S=4096, D=64) on 8 Trainium2 NeuronCores.

Sharding: batch*head parallel. Core c owns flat heads {2c, 2c+1} (same batch
index b = c // 4 for both, so the [S, S] mask is shared by both heads of a
core).

Device algorithm (per core, per head), transposed so no on-chip transposes are
ever needed; the exp() of the softmax is SPLIT between the ScalarE (true exp)
and the VectorE (Schraudolph bit-trick exp) because ScalarE's 1 elem/lane/cycle
throughput on 33.5M elements/core (~219us) is otherwise the hard bottleneck:

  - Host supplies K^T augmented with a ones row as [65, S] fp16, Q^T pre-scaled
    by A*SCALE with a B row as [65, S] fp16, so the score matmul directly
    produces y[k, q] = A*x + B in PSUM, where x = (q . k)/sqrt(D) is the true
    logit, A = 1024/ln2 and B = 1024*(15 - c_rms). V is augmented with a ones
    column as [S, 65] fp16 (row 64 of the AV output = softmax denominator) and
    laid out chunk-major per partition so its DMA is contiguous.
  - Scores are computed transposed: y[k, q] via matmul(lhsT=K^T chunk [65,128],
    rhs=Q^T block [65, 512]); fp16 streams 1 column/cycle and keeps the HAM
    clock at 2.4 GHz.
  - Per score tile (a (3-chunk group, head) pair), one of two drain paths:
      ACT path: ScalarE activation computes pt = exp(y/A - B/A) = exp(x)
        (PSUM->SBUF fp16), then VectorE multiplies by the keep-mask (fp16 2x).
      DVE path (~27% of tiles): one fused VectorE tensor_mul with int16 output:
        i16 = convert(y * maskT). Bit-cast as fp16 this is Schraudolph's
        approximate exp (rel err ~1.7% RMS), and masked entries are exactly
        0x0000 = 0.0. One 1x-rate pass does drain+mask+exp, freeing ScalarE.
    No max-subtraction is needed: scores ~ N(0,1), exp stays in range.
  - AV accumulates transposed-free: matmul(lhsT=[V|1] chunk [128,65],
    rhs=P^T chunk [128,512], fp16) accumulates out^T[d,q] over the 32 k-chunks
    in PSUM; row 64 = softmax denominator. Each slot's AV matmuls are emitted
    AFTER the next slot's score matmuls (PE executes matmuls strictly in
    order, and AV depends on the softmax chain: emitting AV one slot late
    keeps the score stream ahead of ScalarE/VectorE at q-block boundaries).
  - Finished AV accumulators go PSUM -> DRAM directly by DMA; the host divides
    rows 0:64 by row 64 and transposes to [S, 64] during unshard.
"""

from contextlib import ExitStack

import numpy as np

import concourse.tile as tile
from concourse import bacc, mybir
from concourse.bass_utils import run_bass_kernel_spmd

B, H, S, D = 2, 8, 4096, 64
N_CORES = 8
HPC = (B * H) // N_CORES  # heads per core = 2
SCALE = 1.0 / 8.0  # 1/sqrt(D)

# Schraudolph constants for fp16 (10-bit mantissa, bias 15):
#   i16 = round(A*x + B); bitcast(i16) ~= exp(x), rel err ~1.7% RMS.
# A = 1024/ln2; B = 1024*(15 - c) with c ~= 0.0573 (RMS-optimal), rounded so
# B is exactly representable in fp16 (it is sent as a Q^T row).
A_CONST = 1477.3197218702985
B_CONST = 15304.0

F32 = mybir.dt.float32
BF16 = mybir.dt.bfloat16
F16 = mybir.dt.float16
I16 = mybir.dt.int16


def build_kernel_body(tc, qT, kT, vaug, maskT, outT, s=S, hpc=HPC, qb_size=512,
                      group_size=2, psum_s_bufs=3, pt_bufs=8, mask_bufs=8,
                      dve_period=18, dve_slots=(1, 5, 9, 12, 16),
                      gp_tt_every=0, mask_lookahead=4, av_defer=2):
    """Emit the attention program. All APs are DRAM tensors:
    qT, kT: [hpc, 65, s] f16; vaug: [hpc, 128, n_chunks*65] f16;
    maskT: [s, s] f16; outT: [hpc, 65, s] f32.
    """
    nc = tc.nc
    n_qb = s // qb_size
    n_chunks = s // 128
    groups = []
    c0 = 0
    while c0 < n_chunks:
        groups.append((c0, min(group_size, n_chunks - c0)))
        c0 += group_size

    ctx = ExitStack()
    const = ctx.enter_context(tc.tile_pool(name="const", bufs=1))
    mask_pool = ctx.enter_context(tc.tile_pool(name="mask", bufs=mask_bufs))
    pt_pool = ctx.enter_context(tc.tile_pool(name="pt", bufs=pt_bufs))
    out_pool = ctx.enter_context(tc.tile_pool(name="osb", bufs=4))
    psum_s_pool = ctx.enter_context(
        tc.tile_pool(name="psum_s", bufs=psum_s_bufs, space="PSUM"))
    psum_av_pool = ctx.enter_context(
        tc.tile_pool(name="psum_av", bufs=hpc, space="PSUM"))

    # Resident tensors: Q^T, K^T (fp16, 65 rows: d + affine row), V|1 chunked.
    qT_sb = const.tile([D + 1, hpc, s], F16)
    kT_sb = const.tile([D + 1, hpc, s], F16)
    vaug_sb = const.tile([128, hpc, n_chunks, D + 1], F16)
    # Per-partition bias for the ACT path: exp(y/A - B/A) = exp(x).
    bias_sb = const.tile([128, 1], F32)
    nc.gpsimd.memset(bias_sb[:, :], -B_CONST / A_CONST)
    # Dummy 1-element exp so the ~2.7us ACT_TABLE_LOAD happens during the
    # prologue DMA debt instead of delaying the first real exp.
    tl_sb = const.tile([128, 1], F32)
    nc.scalar.activation(tl_sb[:, :], bias_sb[:, :],
                         mybir.ActivationFunctionType.Exp)

    # Prologue DMAs, ordered and split by first-use time: per-head prefixes
    # feed the first score matmuls; mask tiles feed the first drains; V feeds
    # the first AV groups; qT remainders are only needed from q-block 1
    # (~30us in), so they go last.
    g0w = groups[0][1] * 128
    n_pre_masks = min(6, len(groups))
    for h in range(hpc):
        nc.sync.dma_start(out=kT_sb[:, h, 0:g0w], in_=kT[h, :, 0:g0w])
        nc.sync.dma_start(out=qT_sb[:, h, 0:qb_size], in_=qT[h, :, 0:qb_size])

    # Masks stream on the GpSimd software-DGE queue so their issue ops never
    # serialize behind the big const loads on the Sync queue. A lookahead of
    # `mask_lookahead` groups keeps transfers ahead of consumption even when
    # GpSimd also runs offloaded mask-multiplies.
    mask_plan = []  # (qb, gi) in consumption order
    for qb_ in range(n_qb):
        for gi_ in range(len(groups)):
            mask_plan.append((qb_, gi_))
    mask_tiles = {}
    mask_next = [0]

    def issue_masks(upto):
        while mask_next[0] < min(upto, len(mask_plan)):
            qb_, gi_ = mask_plan[mask_next[0]]
            c0_, gs_ = groups[gi_]
            qs_ = slice(qb_ * qb_size, (qb_ + 1) * qb_size)
            mt = mask_pool.tile([128, group_size, qb_size], F16)
            nc.gpsimd.dma_start(
                out=mt[:, :gs_, :],
                in_=maskT[c0_ * 128:(c0_ + gs_) * 128, qs_].rearrange(
                    "(c p) q -> p c q", p=128),
            )
            mask_tiles[(qb_, gi_)] = mt
            mask_next[0] += 1

    # HAM warm-up: ~12 fp16 matmuls on a memset tile, needing no DMA — they
    # span the prologue DMA debt and bring the PE clock to 2.4 GHz before the
    # first real score matmul issues.
    warm = const.tile([128, qb_size], F16)
    nc.vector.memset(warm, 0.0)
    wp = psum_s_pool.tile([128, group_size, qb_size], F32, name="wp", tag="ps")
    for _ in range(12):
        nc.tensor.matmul(wp[:, 0, :], lhsT=warm[:, 0:128], rhs=warm[:, :],
                         start=True, stop=True)

    issue_masks(n_pre_masks)
    # Const loads in consumption order on the Sync queue: small V prefixes
    # (the first AV groups are on the PE critical path within ~4 slots), kT
    # front half, V remainder, kT back half. The qT remainders (only needed
    # from q-block 1, ~30us in) are issued from inside the slot loop so they
    # never compete with the early mask stream.
    vaug_r = [vaug[h, :, :].rearrange("p (c w) -> p c w", w=D + 1)
              for h in range(hpc)]
    vpre = 2 * group_size
    for h in range(hpc):
        nc.sync.dma_start(out=vaug_sb[:, h, 0:vpre, :],
                          in_=vaug_r[h][:, 0:vpre, :])
    kmid = s // 2
    for h in range(hpc):
        nc.sync.dma_start(out=kT_sb[:, h, g0w:kmid], in_=kT[h, :, g0w:kmid])
    for h in range(hpc):
        nc.sync.dma_start(out=vaug_sb[:, h, vpre:, :],
                          in_=vaug_r[h][:, vpre:, :])
    for h in range(hpc):
        nc.sync.dma_start(out=kT_sb[:, h, kmid:], in_=kT[h, :, kmid:])

    # Flat slot schedule: (qb, group, head). AV for slot i is emitted during
    # slot i+1, after that slot's score matmuls.
    slots = []
    for qb in range(n_qb):
        for gi, (c0_, gs_) in enumerate(groups):
            for h in range(hpc):
                slots.append((qb, gi, c0_, gs_, h))

    av_cur = {}  # h -> (tile, qb, qs)

    def flush_av(h):
        # Drain a finished accumulator: PSUM -> SBUF (alternating engines to
        # balance the two near-critical drain engines), then DMA to DRAM.
        avt, _, qs_ = av_cur[h]
        osb = out_pool.tile([D + 1, qb_size], F32, name="osb")
        if h == 0:
            nc.vector.tensor_copy(osb[:, :], avt[:, :])
        else:
            nc.scalar.copy(osb[:, :], avt[:, :])
        nc.sync.dma_start(out=outT[h, :, qs_], in_=osb[:, :])

    def emit_av(qb, c0_, gs_, h, pt, qs):
        cur = av_cur.get(h)
        if cur is None or cur[1] != qb:
            if cur is not None:
                flush_av(h)
            avt = psum_av_pool.tile([D + 1, qb_size], F32, tag="av", name="av")
            av_cur[h] = (avt, qb, qs)
        avt = av_cur[h][0]
        for j in range(gs_):
            c = c0_ + j
            nc.tensor.matmul(
                avt[:, :],
                lhsT=vaug_sb[:, h, c, :],
                rhs=pt[:, j, :],
                start=(c == 0),
                stop=(c == n_chunks - 1),
            )

    deferred = []
    pending_tt = []
    act_count = 0
    for si, (qb, gi, c0_, gs_, h) in enumerate(slots):
        qs = slice(qb * qb_size, (qb + 1) * qb_size)
        if h == 0:
            gidx = qb * len(groups) + gi
            issue_masks(gidx + 1 + mask_lookahead)
            if si == 20:
                for h_ in range(hpc):
                    if qb_size < s:
                        nc.sync.dma_start(out=qT_sb[:, h_, qb_size:],
                                          in_=qT[h_, :, qb_size:])
        mt = mask_tiles[(qb, gi)]

        ps = psum_s_pool.tile([128, group_size, qb_size], F32)
        for j in range(gs_):
            c = c0_ + j
            nc.tensor.matmul(
                ps[:, j, :],
                lhsT=kT_sb[:, h, c * 128:(c + 1) * 128],
                rhs=qT_sb[:, h, qs],
                start=True,
                stop=True,
            )

        pt = pt_pool.tile([128, group_size, qb_size], F16)
        is_dve = (si % dve_period) in dve_slots
        if is_dve:
            # Fused drain+mask+exp on VectorE: i16 = convert(y*mask); the fp16
            # bit pattern of i16 = round(A*x+B) approximates exp(x); mask=0
            # gives exactly 0.0. Emitted ahead of the previous ACT slot's
            # mask-multiply so it runs concurrently with that ACTIVATE on the
            # in-order Vector queue.
            nc.vector.tensor_mul(
                pt[:, :gs_, :].bitcast(I16), ps[:, :gs_, :], mt[:, :gs_, :])
        else:
            nc.scalar.activation(
                pt[:, :gs_, :], ps[:, :gs_, :],
                mybir.ActivationFunctionType.Exp,
                scale=1.0 / A_CONST, bias=bias_sb[:, :],
            )
        # Previous ACT slot's mask-multiply: deferred one slot so this slot's
        # fused DVE drain (if any) sits ahead of it on the Vector queue.
        if pending_tt:
            opt, omt, ogs, use_gp = pending_tt.pop()
            eng = nc.gpsimd if use_gp else nc.vector
            eng.tensor_mul(opt[:, :ogs, :], opt[:, :ogs, :], omt[:, :ogs, :])
        if not is_dve:
            use_gp = bool(gp_tt_every) and (
                act_count % gp_tt_every == gp_tt_every - 1)
            act_count += 1
            pending_tt.append((pt, mt, gs_, use_gp))

        deferred.append((qb, c0_, gs_, h, pt, qs))
        if len(deferred) > av_defer:
            emit_av(*deferred.pop(0))
    if pending_tt:
        opt, omt, ogs, use_gp = pending_tt.pop()
        eng = nc.gpsimd if use_gp else nc.vector
        eng.tensor_mul(opt[:, :ogs, :], opt[:, :ogs, :], omt[:, :ogs, :])
    while deferred:
        emit_av(*deferred.pop(0))
    for h in range(hpc):
        flush_av(h)
    ctx.close()


def build_nc(s=S, hpc=HPC, **kwargs):
    nc = bacc.Bacc(
        "TRN2",
        target_bir_lowering=False,
        debug=False,
        num_devices=N_CORES,
    )
    n_chunks = s // 128
    qT = nc.dram_tensor("qT", [hpc, D + 1, s], F16, kind="ExternalInput").ap()
    kT = nc.dram_tensor("kT", [hpc, D + 1, s], F16, kind="ExternalInput").ap()
    vaug = nc.dram_tensor(
        "vaug", [hpc, 128, n_chunks * (D + 1)], F16, kind="ExternalInput").ap()
    maskT = nc.dram_tensor("maskT", [s, s], F16, kind="ExternalInput").ap()
    outT = nc.dram_tensor("outT", [hpc, D + 1, s], F32, kind="ExternalOutput").ap()
    with tile.TileContext(nc) as tc:
        build_kernel_body(tc, qT, kT, vaug, maskT, outT, s=s, hpc=hpc, **kwargs)
    nc.compile()
    return nc


_NC_CACHE = {}


def get_nc(**kwargs):
    key = tuple(sorted(kwargs.items()))
    if key not in _NC_CACHE:
        _NC_CACHE[key] = build_nc(**kwargs)
    return _NC_CACHE[key]


def make_in_maps(query, key, value, self_attn_mask):
    """Host-side shard + layout prep. Returns list of 8 per-core input dicts."""
    q = np.asarray(query, dtype=np.float32)
    k = np.asarray(key, dtype=np.float32)
    v = np.asarray(value, dtype=np.float32)
    m = np.asarray(self_attn_mask)
    n_chunks = S // 128
    in_maps = []
    ones = np.ones((S, 1), np.float32)
    qscale = np.float32(A_CONST * SCALE)
    for core in range(N_CORES):
        flats = [HPC * core + i for i in range(HPC)]
        pairs = [(f // H, f % H) for f in flats]
        b = pairs[0][0]
        qT = np.stack([
            np.concatenate([q[b_, h_].T * qscale,
                            np.full((1, S), B_CONST, np.float32)], axis=0)
            for b_, h_ in pairs]).astype(np.float16)
        kT = np.stack([
            np.concatenate([k[b_, h_].T, np.ones((1, S), np.float32)], axis=0)
            for b_, h_ in pairs]).astype(np.float16)
        # [S,65] -> chunk-major per partition: [128, n_chunks*65] contiguous.
        vaug = np.stack([
            np.concatenate([v[b_, h_], ones], axis=1)
            .reshape(n_chunks, 128, D + 1).transpose(1, 0, 2)
            .reshape(128, n_chunks * (D + 1))
            for b_, h_ in pairs]).astype(np.float16)
        maskT = np.ascontiguousarray(
            (~m[b, 0]).T).astype(np.float16)
        in_maps.append({
            "qT": np.ascontiguousarray(qT),
            "kT": np.ascontiguousarray(kT),
            "vaug": np.ascontiguousarray(vaug),
            "maskT": maskT,
        })
    return in_maps


def gather_output(results):
    out = np.empty((B, H, S, D), np.float32)
    for core, r in enumerate(results):
        oT = r["outT"].astype(np.float32)  # [HPC, 65, S]
        for i in range(HPC):
            f = HPC * core + i
            b_, h_ = f // H, f % H
            out[b_, h_] = (oT[i, :D, :] / oT[i, D:D + 1, :]).T
    return out


def kernel(query, key, value, self_attn_mask, trace=False, tmpdir=None,
           **build_kwargs):
    nc = get_nc(**build_kwargs)
    in_maps = make_in_maps(query, key, value, self_attn_mask)
    kwargs = {"tmpdir": tmpdir} if tmpdir else {}
    res = run_bass_kernel_spmd(nc, in_maps, core_ids=list(range(N_CORES)),
                               trace=trace, **kwargs)
    out = gather_output(res.results)
    if trace:
        kernel.last_result = res
    return out
